# revision 26
# baseline (speedup 1.0000x reference)
"""Trainium2 Bass kernel for segment-mean + linear head + L2-normalize.

Reference computation (per batch element b, frame t):
  mean[s, c]  = mean over pixels p with sp_mask[p] == s of maps[c, p]
  sp[s, d]    = sum_c mean[s, c] * W_fc[d, c]
  out[d, s]   = sp[s, d] / max(||sp[s, :]||_2, 1e-12)

Key identities/choices:
- the per-segment count division cancels inside the L2 normalization
  (normalize(v / n) == normalize(v) for n > 0, and empty segments are
  exactly zero either way), so counts are never computed.
- maps are pre-cast to float8_e3m4 on the host (4 mantissa bits): the
  kernel is HBM-bound, fp8 halves read volume vs bf16. Measured end-to-end
  rel err 1.40e-2 (vs 3.0e-3 at bf16) on the seed-0 inputs.
- data-parallel over B: one clip per NeuronCore, 8 cores.

Active variant "pkbf" per clip (see _build_program_pkbf):
  1. host relayouts maps to (P, K*T*C) fp8, pixel index = k*128 + p; the
     clip is DMAed in 8 x 1MB chunks alternating the two HWDGE rings.
  2. one-hot O[p, (k,s)] = (sp_mask==s) built on DVE (iota + is_equal),
     pipelined one rep ahead.
  3. stage A (PE): segc[s, c] += O_kt[p, s]^T @ feats[p, (k,t,:)] -- the
     one-hot is the STATIONARY operand, feats stream as the moving operand
     (N=512). 128 fat matmuls/clip. This matters: LDWEIGHTS runs at
     ~1 col/1.2GHz-cycle (no FWL on this toolchain), so thin matmuls
     (feats stationary, N=100) are LDWEIGHTS-bound at ~107ns -> 55us/clip,
     while fat ones hide the 83ns weight load under the 213ns stream and
     approach the 65.5k-cycle (27us) feats-through-PE floor.
  4. stage B (PE): 4 transposes of segc per frame + project through W^T,
     PSUM-bank layout: 4 segc banks + 2 seg + 2 transpose staging = 8.
  5. norm epilogue on DVE (+4 ACT sqrts), one gpsimd output DMA per clip.

Measured (8 cores in parallel, marginal two-slope method, clean window):
  full ~30.6-33.8us/clip, compute-only ~34.8us, fp8 DMA floor ~21.5us
  (vs 87.4us graded baseline at bf16 with thin matmuls). oall_bufs=8 is
  load-bearing: with 6, next-rep one-hot builds wait on this rep's whole
  stage A and the boundary stall serializes DMA behind compute (~3x).
"""

import numpy as np

B, C, T, H, W = 8, 512, 4, 64, 64
HW = H * W          # 4096 pixels per frame
N_SP = 100
D_OUT = 128
N_CORES = 8
P = 128             # SBUF partitions
NCH = C // P        # 4 channel chunks
NPIX = HW // P      # 32 pixel chunks

_CACHE = {}


def _build_program(reps=1, feats_bufs=3, dma_split=2, dma_only=False, pp_bufs=3, proj_bufs=4, compute_only=False):
    from contextlib import ExitStack

    import concourse.tile as tile
    from concourse import bacc, mybir
    from concourse.masks import make_identity

    f32 = mybir.dt.float32
    bf16 = mybir.dt.bfloat16
    i32 = mybir.dt.int32

    nc = bacc.Bacc(
        "TRN2",
        target_bir_lowering=False,
        debug=False,
        num_devices=N_CORES,
    )

    # maps and W arrive pre-cast to bf16 (host-side) -- the kernel computes in
    # bf16 anyway, so this halves the HBM read volume at identical accuracy
    maps_t = nc.dram_tensor("maps_bf16", [C, T, H, W], bf16, kind="ExternalInput")
    mask_t = nc.dram_tensor("sp_mask", [T, H, W], i32, kind="ExternalInput")
    wfc_t = nc.dram_tensor("W_fcT", [C, D_OUT], bf16, kind="ExternalInput")
    out_t = nc.dram_tensor("out", [T, N_SP, D_OUT], f32, kind="ExternalOutput")

    # DRAM views
    # maps as (c_in_chunk, chunk, t, pixel): partition dim = channel-in-chunk
    maps_r = maps_t.ap().rearrange("(cj c) t h w -> c cj t (h w)", c=P)
    # sp_mask frame as (chunk, pixel_in_chunk): chunk k = pixels [128k, 128k+128)
    mask_r = mask_t.ap().rearrange("t h w -> t (h w)").rearrange(
        "t (k p) -> t k p", p=P
    )
    # W_fc^T per channel chunk: (c_in_chunk, chunk, d)
    wfc_r = wfc_t.ap().rearrange("(cj c) d -> c cj d", c=P)

    with tile.TileContext(nc) as tc, ExitStack() as ctx:
        const_pool = ctx.enter_context(tc.tile_pool(name="const", bufs=1))
        feats_pool = ctx.enter_context(tc.tile_pool(name="feats", bufs=feats_bufs))
        mask_pool = ctx.enter_context(tc.tile_pool(name="mask", bufs=4))
        maskf_pool = ctx.enter_context(tc.tile_pool(name="maskf", bufs=4))
        oall_pool = ctx.enter_context(tc.tile_pool(name="oall", bufs=4))
        proj_pool = ctx.enter_context(tc.tile_pool(name="proj", bufs=proj_bufs))
        outsb_pool = ctx.enter_context(tc.tile_pool(name="outsb", bufs=2))
        small_pool = ctx.enter_context(tc.tile_pool(name="small", bufs=4))
        pp_pool = ctx.enter_context(tc.tile_pool(name="pp", bufs=pp_bufs, space="PSUM"))
        seg_pool = ctx.enter_context(tc.tile_pool(name="seg", bufs=2, space="PSUM"))
        mtp_pool = ctx.enter_context(tc.tile_pool(name="mtp", bufs=2, space="PSUM"))

        # iota constant: column k*100+s holds value s (segment id pattern).
        # Values are 0..99, exact in f32.
        iota_tile = const_pool.tile([P, NPIX * N_SP], f32)
        nc.gpsimd.iota(
            iota_tile[:].rearrange("p (k s) -> p k s", s=N_SP),
            pattern=[[0, NPIX], [1, N_SP]],
            base=0,
            channel_multiplier=0,
            allow_small_or_imprecise_dtypes=True,
        )

        identity = const_pool.tile([P, P], f32)
        make_identity(nc, identity[:])

        # tiny positive bias so sqrt(ss + eps) never hits 1/0 on empty segments
        eps_tile = const_pool.tile([P, 1], f32)
        nc.vector.memset(eps_tile[:], 1e-30)

        # W^T in SBUF as bf16: column block cj holds (c_in_chunk, d) for chunk cj
        wt_tile = const_pool.tile([P, NCH * D_OUT], bf16)
        nc.sync.dma_start(
            out=wt_tile[:].rearrange("c (cj d) -> c cj d", d=D_OUT),
            in_=wfc_r,
        )

        static_feats = None
        if compute_only:
            static_feats = const_pool.tile([P, NCH * HW], bf16)
            nc.sync.dma_start(
                out=static_feats[:].rearrange("c (cj p) -> c cj p", p=HW),
                in_=maps_r[:, :, 0, :],
            )

        # prologue: build all four frames' one-hot matrices up front -- they
        # depend only on the tiny sp_mask tensor, so this runs entirely under
        # the first feats DMA instead of on each frame's critical path
        oalls = []
        if not dma_only:
            for t in range(T):
                mask_raw = mask_pool.tile([NPIX, P], f32)
                nc.gpsimd.dma_start(out=mask_raw[:], in_=mask_r[t])
                mask_ps = mtp_pool.tile([P, NPIX], f32)
                nc.tensor.transpose(
                    out=mask_ps[:], in_=mask_raw[:], identity=identity[:NPIX, :NPIX]
                )
                mask_f = maskf_pool.tile([P, NPIX], f32)
                nc.scalar.copy(out=mask_f[:], in_=mask_ps[:])
                oall = oall_pool.tile([P, NPIX * N_SP], bf16)
                nc.vector.tensor_tensor(
                    out=oall[:].rearrange("p (k s) -> p k s", s=N_SP),
                    in0=mask_f[:].to_broadcast([P, NPIX, N_SP]),
                    in1=iota_tile[:].rearrange("p (k s) -> p k s", s=N_SP),
                    op=mybir.AluOpType.is_equal,
                )
                oalls.append(oall)

        for t in [t for _ in range(reps) for t in range(T)]:
            if compute_only:
                feats = static_feats
            else:
                feats = feats_pool.tile([P, NCH * HW], bf16)
            for sp_i in range(0 if compute_only else dma_split):
                lo = sp_i * (NCH // dma_split)
                hi = (sp_i + 1) * (NCH // dma_split)
                dma_eng = nc.scalar if (sp_i % 2 == 1) else nc.sync
                dma_eng.dma_start(
                    out=feats[:, lo * HW : hi * HW].rearrange(
                        "c (cj p) -> c cj p", p=HW
                    ),
                    in_=maps_r[:, lo:hi, t, :],
                )

            if dma_only:
                # measure the pure input-DMA floor: touch feats with one tiny
                # op per frame so the loads stay live, skip all compute
                probe = small_pool.tile([P, 1], f32)
                nc.vector.reduce_sum(
                    out=probe[:], in_=feats[:, :4], axis=mybir.AxisListType.X
                )
                continue

            oall = oalls[t]
            seg = seg_pool.tile([N_SP, D_OUT], f32)
            for g in range(NPIX // 4):
                pp = pp_pool.tile([P, 4 * D_OUT], f32)
                for i in range(4):
                    ch = g * 4 + i
                    for cj in range(NCH):
                        nc.tensor.matmul(
                            out=pp[:, i * D_OUT : (i + 1) * D_OUT],
                            lhsT=feats[:, cj * HW + ch * P : cj * HW + (ch + 1) * P],
                            rhs=wt_tile[:, cj * D_OUT : (cj + 1) * D_OUT],
                            start=(cj == 0),
                            stop=(cj == NCH - 1),
                        )
                proj = proj_pool.tile([P, 4 * D_OUT], bf16)
                nc.scalar.copy(out=proj[:], in_=pp[:])
                for i in range(4):
                    ch = g * 4 + i
                    nc.tensor.matmul(
                        out=seg[:],
                        lhsT=oall[:, ch * N_SP : (ch + 1) * N_SP],
                        rhs=proj[:, i * D_OUT : (i + 1) * D_OUT],
                        start=(ch == 0),
                        stop=(ch == NPIX - 1),
                    )

            sq = small_pool.tile([N_SP, D_OUT], f32)
            ss = small_pool.tile([N_SP, 1], f32)
            nc.scalar.activation(
                out=sq[:],
                in_=seg[:],
                func=mybir.ActivationFunctionType.Square,
                accum_out=ss[:],
            )
            nrm = small_pool.tile([N_SP, 1], f32)
            nc.scalar.activation(
                out=nrm[:],
                in_=ss[:],
                func=mybir.ActivationFunctionType.Sqrt,
                bias=eps_tile[:N_SP],
            )
            inv = small_pool.tile([N_SP, 1], f32)
            nc.vector.reciprocal(out=inv[:], in_=nrm[:])
            outsb = outsb_pool.tile([N_SP, D_OUT], f32)
            nc.vector.tensor_scalar_mul(out=outsb[:], in0=seg[:], scalar1=inv[:])
            nc.sync.dma_start(out=out_t.ap()[t], in_=outsb[:])

    nc.compile()
    return nc


def _build_program_pm(reps=1, feats_bufs=3, dma_split=2, dma_only=False):
    """Pixel-major variant: host supplies maps as (T, HW, C) bf16.

    Per frame: one-hot segment-sum over raw features first
    (seg_c[s, c] = sum_p O[p, s] * feats[p, c], 32 matmuls of N=512),
    then project the tiny (100, 512) result through W^T (4 matmuls of
    N=128 after 4 PE transposes). ~210M MACs/frame vs 320M for the
    projection-first ordering, and no big PSUM->SBUF copy traffic.
    """
    from contextlib import ExitStack

    import concourse.tile as tile
    from concourse import bacc, mybir
    from concourse.masks import make_identity

    f32 = mybir.dt.float32
    bf16 = mybir.dt.bfloat16
    i32 = mybir.dt.int32

    nc = bacc.Bacc(
        "TRN2",
        target_bir_lowering=False,
        debug=False,
        num_devices=N_CORES,
    )

    maps_t = nc.dram_tensor("maps_pm", [T, HW, C], bf16, kind="ExternalInput")
    mask_t = nc.dram_tensor("sp_mask", [T, H, W], i32, kind="ExternalInput")
    wfc_t = nc.dram_tensor("W_fcT", [C, D_OUT], bf16, kind="ExternalInput")
    out_t = nc.dram_tensor("out", [T, N_SP, D_OUT], f32, kind="ExternalOutput")

    # (pixel_in_chunk, chunk, t, channel)
    maps_r = maps_t.ap().rearrange("t (k p) c -> p k t c", p=P)
    mask_r = mask_t.ap().rearrange("t h w -> t (h w)").rearrange(
        "t (k p) -> t k p", p=P
    )
    wfc_r = wfc_t.ap().rearrange("(cj c) d -> c cj d", c=P)

    with tile.TileContext(nc) as tc, ExitStack() as ctx:
        const_pool = ctx.enter_context(tc.tile_pool(name="const", bufs=1))
        feats_pool = ctx.enter_context(tc.tile_pool(name="feats", bufs=feats_bufs))
        mask_pool = ctx.enter_context(tc.tile_pool(name="mask", bufs=4))
        maskf_pool = ctx.enter_context(tc.tile_pool(name="maskf", bufs=4))
        oall_pool = ctx.enter_context(tc.tile_pool(name="oall", bufs=4))
        segsb_pool = ctx.enter_context(tc.tile_pool(name="segsb", bufs=2))
        ctsb_pool = ctx.enter_context(tc.tile_pool(name="ctsb", bufs=2))
        outsb_pool = ctx.enter_context(tc.tile_pool(name="outsb", bufs=2))
        small_pool = ctx.enter_context(tc.tile_pool(name="small", bufs=4))
        segc_pool = ctx.enter_context(tc.tile_pool(name="segc", bufs=2, space="PSUM"))
        ct_pool = ctx.enter_context(tc.tile_pool(name="ct", bufs=2, space="PSUM"))
        seg_pool = ctx.enter_context(tc.tile_pool(name="seg", bufs=2, space="PSUM"))
        mtp_pool = ctx.enter_context(tc.tile_pool(name="mtp", bufs=2, space="PSUM"))

        # iota constant (bf16: values 0..99 are exact)
        iota_tile = const_pool.tile([P, NPIX * N_SP], bf16)
        nc.gpsimd.iota(
            iota_tile[:].rearrange("p (k s) -> p k s", s=N_SP),
            pattern=[[0, NPIX], [1, N_SP]],
            base=0,
            channel_multiplier=0,
            allow_small_or_imprecise_dtypes=True,
        )

        identity = const_pool.tile([P, P], bf16)
        make_identity(nc, identity[:])

        eps_tile = const_pool.tile([P, 1], f32)
        nc.vector.memset(eps_tile[:], 1e-30)

        wt_tile = const_pool.tile([P, NCH * D_OUT], bf16)
        nc.sync.dma_start(
            out=wt_tile[:].rearrange("c (cj d) -> c cj d", d=D_OUT),
            in_=wfc_r,
        )

        for t in [t for _ in range(reps) for t in range(T)]:
            feats = feats_pool.tile([P, NPIX * C], bf16)
            for sp_i in range(dma_split):
                lo = sp_i * (NPIX // dma_split)
                hi = (sp_i + 1) * (NPIX // dma_split)
                dma_eng = nc.scalar if (sp_i % 2 == 1) else nc.sync
                dma_eng.dma_start(
                    out=feats[:, lo * C : hi * C].rearrange(
                        "p (k c) -> p k c", c=C
                    ),
                    in_=maps_r[:, lo:hi, t, :],
                )

            if dma_only:
                probe = small_pool.tile([P, 1], f32)
                nc.vector.reduce_sum(
                    out=probe[:], in_=feats[:, :4], axis=mybir.AxisListType.X
                )
                continue

            # mask: (chunk, pixel) i32 -> bf16, PE-transpose to (pixel, chunk)
            mask_raw = mask_pool.tile([NPIX, P], bf16)
            nc.gpsimd.dma_start(out=mask_raw[:], in_=mask_r[t])
            mask_ps = mtp_pool.tile([P, NPIX], bf16)
            nc.tensor.transpose(
                out=mask_ps[:], in_=mask_raw[:], identity=identity[:NPIX, :NPIX]
            )
            mask_f = maskf_pool.tile([P, NPIX], bf16)
            nc.scalar.copy(out=mask_f[:], in_=mask_ps[:])

            oall = oall_pool.tile([P, NPIX * N_SP], bf16)
            nc.vector.tensor_tensor(
                out=oall[:].rearrange("p (k s) -> p k s", s=N_SP),
                in0=mask_f[:].to_broadcast([P, NPIX, N_SP]),
                in1=iota_tile[:].rearrange("p (k s) -> p k s", s=N_SP),
                op=mybir.AluOpType.is_equal,
            )

            # stage A: per-segment channel sums, seg_c (100 s, 512 c)
            segc = segc_pool.tile([N_SP, C], f32)
            for k in range(NPIX):
                nc.tensor.matmul(
                    out=segc[:],
                    lhsT=oall[:, k * N_SP : (k + 1) * N_SP],
                    rhs=feats[:, k * C : (k + 1) * C],
                    start=(k == 0),
                    stop=(k == NPIX - 1),
                )
            segc_sb = segsb_pool.tile([N_SP, C], bf16)
            nc.scalar.copy(out=segc_sb[:], in_=segc[:])

            # transpose seg_c -> (c, s) per channel chunk, then project
            ct_sb = ctsb_pool.tile([P, NCH * N_SP], bf16)
            for cj in range(NCH):
                ctp = ct_pool.tile([P, N_SP], bf16)
                nc.tensor.transpose(
                    out=ctp[:],
                    in_=segc_sb[:, cj * P : (cj + 1) * P],
                    identity=identity[:N_SP, :N_SP],
                )
                nc.scalar.copy(
                    out=ct_sb[:, cj * N_SP : (cj + 1) * N_SP], in_=ctp[:]
                )

            # stage B: seg (100 s, 128 d) = seg_c @ W^T
            seg = seg_pool.tile([N_SP, D_OUT], f32)
            for cj in range(NCH):
                nc.tensor.matmul(
                    out=seg[:],
                    lhsT=ct_sb[:, cj * N_SP : (cj + 1) * N_SP],
                    rhs=wt_tile[:, cj * D_OUT : (cj + 1) * D_OUT],
                    start=(cj == 0),
                    stop=(cj == NCH - 1),
                )

            sq = small_pool.tile([N_SP, D_OUT], f32)
            ss = small_pool.tile([N_SP, 1], f32)
            nc.scalar.activation(
                out=sq[:],
                in_=seg[:],
                func=mybir.ActivationFunctionType.Square,
                accum_out=ss[:],
            )
            nrm = small_pool.tile([N_SP, 1], f32)
            nc.scalar.activation(
                out=nrm[:],
                in_=ss[:],
                func=mybir.ActivationFunctionType.Sqrt,
                bias=eps_tile[:N_SP],
            )
            inv = small_pool.tile([N_SP, 1], f32)
            nc.vector.reciprocal(out=inv[:], in_=nrm[:])
            outsb = outsb_pool.tile([N_SP, D_OUT], f32)
            nc.vector.tensor_scalar_mul(out=outsb[:], in0=seg[:], scalar1=inv[:])
            nc.sync.dma_start(out=out_t.ap()[t], in_=outsb[:])

    nc.compile()
    return nc


def _build_program_pkb(reps=1, dma_only=False, compute_only=False, nchunk=8,
                       fp8=False, feats_bufs=None, oall_bufs=6):
    """Pixel-blocked variant: host supplies maps as (P, K*T*C) bf16 (or
    float8_e3m4 when fp8=True), where
    column layout is (k, t, c) — pixel index = k*128 + p.

    Design (see measurements in session notes):
    - segment-sum FIRST on raw features with feats-stationary matmuls
      (contraction over pixels on the partition dim): 51.2k PE cycles/clip
      vs 82k for projection-first.
        stage A: segcT[c,s] += feats[p,(k,t,cb)]^T @ onehot[p,(k,s)]
        stage B: seg[s,d] = segcT^T @ W^T per channel block
    - k-major DMA layout gives long contiguous DRAM runs; the clip is split
      into `nchunk` chunk-tiles (16KB runs at nchunk=8) alternating the two
      HWDGE rings, and stage A is chunk-major so compute consumes each chunk
      as it lands and next-rep DMA overlaps this rep's tail compute.
    - ONE matmul `start` per PSUM bank (start marks the whole 2KB zero
      region pending-zero; other cb regions self-initialize on first write).
    - mask -> one-hot prologue for rep r+1 is emitted during rep r, so the
      PE/DVE never stall on it at rep boundaries.
    - epilogue runs on DVE (copies, square+reduce, reciprocal, scale); the
      ACT queue carries only its HWDGE ring triggers + 4 tiny sqrts, so
      next-rep input DMA triggers are not stuck behind epilogue compute.
    - all four frame outputs leave in ONE gpsimd DMA per clip (small per-rep
      DMAs on the input rings were measured to stall them).
    """
    from contextlib import ExitStack

    import concourse.tile as tile
    from concourse import bacc, mybir
    from concourse.masks import make_identity

    f32 = mybir.dt.float32
    bf16 = mybir.dt.bfloat16
    i32 = mybir.dt.int32
    fdt = mybir.dt.float8e3 if fp8 else bf16

    K = NPIX            # 32 k-chunks of 128 pixels
    KTC = K * T * C     # 65536 columns per partition
    CB = 4              # channel blocks of 128
    KPC = K // nchunk   # k-chunks per DMA chunk
    if feats_bufs is None:
        feats_bufs = nchunk

    nc = bacc.Bacc(
        "TRN2",
        target_bir_lowering=False,
        debug=False,
        num_devices=N_CORES,
    )

    maps_t = nc.dram_tensor("maps_pkb", [P, KTC], fdt, kind="ExternalInput")
    mask_t = nc.dram_tensor("sp_mask", [T, H, W], i32, kind="ExternalInput")
    wfc_t = nc.dram_tensor("W_fcT", [C, D_OUT], bf16, kind="ExternalInput")
    out_t = nc.dram_tensor("out", [T, N_SP, D_OUT], f32, kind="ExternalOutput")

    mask_r = mask_t.ap().rearrange("t h w -> t (h w)").rearrange(
        "t (k p) -> t k p", p=P
    )
    wfc_r = wfc_t.ap().rearrange("(cj c) d -> c cj d", c=P)

    with tile.TileContext(nc) as tc, ExitStack() as ctx:
        const_pool = ctx.enter_context(tc.tile_pool(name="const", bufs=1))
        feats_pool = ctx.enter_context(tc.tile_pool(name="feats", bufs=feats_bufs))
        mask_pool = ctx.enter_context(tc.tile_pool(name="mask", bufs=8))
        maskf_pool = ctx.enter_context(tc.tile_pool(name="maskf", bufs=8))
        oall_pool = ctx.enter_context(tc.tile_pool(name="oall", bufs=oall_bufs))
        segsb_pool = ctx.enter_context(tc.tile_pool(name="segsb", bufs=4))
        segfs_pool = ctx.enter_context(tc.tile_pool(name="segfs", bufs=4))
        outsb_pool = ctx.enter_context(tc.tile_pool(name="outsb", bufs=2))
        small_pool = ctx.enter_context(tc.tile_pool(name="small", bufs=4))
        segct_pool = ctx.enter_context(
            tc.tile_pool(name="segct", bufs=1, space="PSUM")
        )
        seg_pool = ctx.enter_context(tc.tile_pool(name="seg", bufs=2, space="PSUM"))
        mtp_pool = ctx.enter_context(tc.tile_pool(name="mtp", bufs=2, space="PSUM"))

        # iota constant: column (k, s) holds value s (bf16: 0..99 exact)
        iota_tile = const_pool.tile([P, K * N_SP], bf16)
        nc.gpsimd.iota(
            iota_tile[:].rearrange("p (k s) -> p k s", s=N_SP),
            pattern=[[0, K], [1, N_SP]],
            base=0,
            channel_multiplier=0,
            allow_small_or_imprecise_dtypes=True,
        )

        identity = const_pool.tile([P, P], bf16)
        make_identity(nc, identity[:])

        eps_tile = const_pool.tile([P, 1], f32)
        nc.vector.memset(eps_tile[:], 1e-30)

        wt_tile = const_pool.tile([P, CB * D_OUT], bf16)
        nc.sync.dma_start(
            out=wt_tile[:].rearrange("c (cj d) -> c cj d", d=D_OUT),
            in_=wfc_r,
        )

        static_chunks = None
        if compute_only:
            static_chunks = []
            for cj in range(nchunk):
                sc = const_pool.tile([P, KPC * T * C], fdt, name=f"static{cj}")
                nc.sync.dma_start(
                    out=sc[:],
                    in_=maps_t.ap()[:, cj * KPC * T * C : (cj + 1) * KPC * T * C],
                )
                static_chunks.append(sc)

        def emit_prologue():
            """mask -> transpose -> one-hot for one rep; returns oall tiles."""
            oalls = []
            for t in range(T):
                mask_raw = mask_pool.tile([K, P], bf16)
                nc.gpsimd.dma_start(out=mask_raw[:], in_=mask_r[t])
                mask_ps = mtp_pool.tile([P, K], bf16)
                nc.tensor.transpose(
                    out=mask_ps[:], in_=mask_raw[:], identity=identity[:K, :K]
                )
                mask_f = maskf_pool.tile([P, K], bf16)
                nc.vector.tensor_copy(out=mask_f[:], in_=mask_ps[:])
                oall = oall_pool.tile([P, K * N_SP], fdt)
                nc.vector.tensor_tensor(
                    out=oall[:].rearrange("p (k s) -> p k s", s=N_SP),
                    in0=mask_f[:].to_broadcast([P, K, N_SP]),
                    in1=iota_tile[:].rearrange("p (k s) -> p k s", s=N_SP),
                    op=mybir.AluOpType.is_equal,
                )
                oalls.append(oall)
            return oalls

        oalls = None if dma_only else emit_prologue()

        for r in range(reps):
            if compute_only:
                chunks = static_chunks
            else:
                chunks = []
                for cj in range(nchunk):
                    ct = feats_pool.tile([P, KPC * T * C], fdt)
                    eng = nc.sync if cj % 2 == 0 else nc.scalar
                    eng.dma_start(
                        out=ct[:],
                        in_=maps_t.ap()[:, cj * KPC * T * C : (cj + 1) * KPC * T * C],
                    )
                    chunks.append(ct)

            if dma_only:
                probe = small_pool.tile([P, 1], f32)
                nc.vector.reduce_sum(
                    out=probe[:], in_=chunks[-1][:, :4], axis=mybir.AxisListType.X
                )
                continue

            cur_oalls = oalls
            if r + 1 < reps:
                oalls = emit_prologue()  # next rep's, built during this rep

            # stage A: chunk-major; ONE start/stop per frame PSUM bank
            segcts = [
                segct_pool.tile([P, CB * N_SP], f32, name=f"segct{t}")
                for t in range(T)
            ]
            for cj in range(nchunk):
                ct = chunks[cj]
                for t in range(T):
                    for kk in range(KPC):
                        k = cj * KPC + kk
                        for cb in range(CB):
                            nc.tensor.matmul(
                                out=segcts[t][:, cb * N_SP : (cb + 1) * N_SP],
                                lhsT=ct[
                                    :,
                                    kk * T * C + t * C + cb * P : kk * T * C
                                    + t * C
                                    + (cb + 1) * P,
                                ],
                                rhs=cur_oalls[t][:, k * N_SP : (k + 1) * N_SP],
                                start=(k == 0 and cb == 0),
                                stop=(k == K - 1 and cb == CB - 1),
                            )

            # stage B (phase-major so engines overlap across frames)
            segsbs, segs, segfs_l = [], [], []
            for t in range(T):
                segct_sb = segsb_pool.tile([P, CB * N_SP], bf16)
                nc.vector.tensor_copy(out=segct_sb[:], in_=segcts[t][:])
                segsbs.append(segct_sb)
            for t in range(T):
                seg = seg_pool.tile([N_SP, D_OUT], f32)
                for cb in range(CB):
                    nc.tensor.matmul(
                        out=seg[:],
                        lhsT=segsbs[t][:, cb * N_SP : (cb + 1) * N_SP],
                        rhs=wt_tile[:, cb * D_OUT : (cb + 1) * D_OUT],
                        start=(cb == 0),
                        stop=(cb == CB - 1),
                    )
                segs.append(seg)
            outsb = outsb_pool.tile([N_SP, T * D_OUT], f32)
            for t in range(T):
                # seg -> SBUF copy frees the PSUM bank after two DVE reads
                seg_fs = segfs_pool.tile([N_SP, D_OUT], f32)
                nc.vector.tensor_copy(out=seg_fs[:], in_=segs[t][:])
                segfs_l.append(seg_fs)
                sq = small_pool.tile([N_SP, D_OUT], f32)
                ss = small_pool.tile([N_SP, 1], f32)
                nc.vector.tensor_mul(out=sq[:], in0=seg_fs[:], in1=seg_fs[:])
                nc.vector.reduce_sum(
                    out=ss[:], in_=sq[:], axis=mybir.AxisListType.X
                )
                nrm = small_pool.tile([N_SP, 1], f32)
                nc.scalar.activation(
                    out=nrm[:],
                    in_=ss[:],
                    func=mybir.ActivationFunctionType.Sqrt,
                    bias=eps_tile[:N_SP],
                )
                inv = small_pool.tile([N_SP, 1], f32)
                nc.vector.reciprocal(out=inv[:], in_=nrm[:])
                nc.vector.tensor_scalar_mul(
                    out=outsb[:, t * D_OUT : (t + 1) * D_OUT],
                    in0=segfs_l[t][:],
                    scalar1=inv[:],
                )
            nc.gpsimd.dma_start(
                out=out_t.ap().rearrange("t s d -> s t d"),
                in_=outsb[:].rearrange("s (t d) -> s t d", d=D_OUT),
            )

    nc.compile()
    return nc


def _build_program_pkbf(reps=1, dma_only=False, compute_only=False, nchunk=8,
                        feats_bufs=None, oall_bufs=8, copy_eng="vector"):
    """Fat-matmul fp8 variant: host supplies maps as (P, K*T*C) float8_e3m4,
    column layout (k, t, c) — pixel index = k*128 + p.

    Stage A makes the ONE-HOT the stationary operand and streams feats as
    the moving operand:
        segc[s, c] += onehot_kt[p, s]^T @ feats[p, (k,t,:)]     (N=512)
    128 matmuls/clip instead of 512. On this hardware LDWEIGHTS runs at
    ~1 col / 1.2GHz-cycle (no FWL) and binds thin matmuls at ~107ns; with
    the fat stream the 83ns one-hot load hides under the 213ns stream, so
    stage A approaches the 65.5k-cycle (27us) stream floor.

    Stage B transposes segc per channel block on the PE (16 small
    transposes) and projects through W^T as in the pm variant.
    """
    from contextlib import ExitStack

    import concourse.tile as tile
    from concourse import bacc, mybir
    from concourse.masks import make_identity

    f32 = mybir.dt.float32
    bf16 = mybir.dt.bfloat16
    i32 = mybir.dt.int32
    fdt = mybir.dt.float8e3

    K = NPIX            # 32 k-chunks of 128 pixels
    KTC = K * T * C     # 65536 columns per partition
    CB = 4              # channel blocks of 128
    KPC = K // nchunk   # k-chunks per DMA chunk
    if feats_bufs is None:
        feats_bufs = nchunk

    nc = bacc.Bacc(
        "TRN2",
        target_bir_lowering=False,
        debug=False,
        num_devices=N_CORES,
    )

    maps_t = nc.dram_tensor("maps_pkb", [P, KTC], fdt, kind="ExternalInput")
    mask_t = nc.dram_tensor("sp_mask", [T, H, W], i32, kind="ExternalInput")
    wfc_t = nc.dram_tensor("W_fcT", [C, D_OUT], bf16, kind="ExternalInput")
    out_t = nc.dram_tensor("out", [T, N_SP, D_OUT], f32, kind="ExternalOutput")

    mask_r = mask_t.ap().rearrange("t h w -> t (h w)").rearrange(
        "t (k p) -> t k p", p=P
    )
    wfc_r = wfc_t.ap().rearrange("(cj c) d -> c cj d", c=P)

    with tile.TileContext(nc) as tc, ExitStack() as ctx:
        const_pool = ctx.enter_context(tc.tile_pool(name="const", bufs=1))
        feats_pool = ctx.enter_context(tc.tile_pool(name="feats", bufs=feats_bufs))
        mask_pool = ctx.enter_context(tc.tile_pool(name="mask", bufs=8))
        maskf_pool = ctx.enter_context(tc.tile_pool(name="maskf", bufs=8))
        oall_pool = ctx.enter_context(tc.tile_pool(name="oall", bufs=oall_bufs))
        segsb_pool = ctx.enter_context(tc.tile_pool(name="segsb", bufs=4))
        ctsb_pool = ctx.enter_context(tc.tile_pool(name="ctsb", bufs=4))
        segfs_pool = ctx.enter_context(tc.tile_pool(name="segfs", bufs=4))
        outsb_pool = ctx.enter_context(tc.tile_pool(name="outsb", bufs=2))
        small_pool = ctx.enter_context(tc.tile_pool(name="small", bufs=4))
        segc_pool = ctx.enter_context(
            tc.tile_pool(name="segc", bufs=1, space="PSUM")
        )
        seg_pool = ctx.enter_context(tc.tile_pool(name="seg", bufs=2, space="PSUM"))
        tp_pool = ctx.enter_context(tc.tile_pool(name="tp", bufs=2, space="PSUM"))

        # iota constant: column (k, s) holds value s (bf16: 0..99 exact)
        iota_tile = const_pool.tile([P, K * N_SP], bf16)
        nc.gpsimd.iota(
            iota_tile[:].rearrange("p (k s) -> p k s", s=N_SP),
            pattern=[[0, K], [1, N_SP]],
            base=0,
            channel_multiplier=0,
            allow_small_or_imprecise_dtypes=True,
        )

        identity = const_pool.tile([P, P], bf16)
        make_identity(nc, identity[:])

        eps_tile = const_pool.tile([P, 1], f32)
        nc.vector.memset(eps_tile[:], 1e-30)

        wt_tile = const_pool.tile([P, CB * D_OUT], bf16)
        nc.sync.dma_start(
            out=wt_tile[:].rearrange("c (cj d) -> c cj d", d=D_OUT),
            in_=wfc_r,
        )

        copy = nc.vector.tensor_copy if copy_eng == "vector" else nc.scalar.copy

        static_chunks = None
        if compute_only:
            static_chunks = []
            for cj in range(nchunk):
                sc = const_pool.tile([P, KPC * T * C], fdt, name=f"static{cj}")
                nc.sync.dma_start(
                    out=sc[:],
                    in_=maps_t.ap()[:, cj * KPC * T * C : (cj + 1) * KPC * T * C],
                )
                static_chunks.append(sc)

        def emit_prologue():
            """mask -> transpose -> one-hot for one rep; returns oall tiles."""
            oalls = []
            for t in range(T):
                mask_raw = mask_pool.tile([K, P], bf16)
                nc.gpsimd.dma_start(out=mask_raw[:], in_=mask_r[t])
                mask_ps = tp_pool.tile([P, K], bf16, name="tp")
                nc.tensor.transpose(
                    out=mask_ps[:], in_=mask_raw[:], identity=identity[:K, :K]
                )
                mask_f = maskf_pool.tile([P, K], bf16)
                nc.vector.tensor_copy(out=mask_f[:], in_=mask_ps[:])
                oall = oall_pool.tile([P, K * N_SP], fdt)
                nc.vector.tensor_tensor(
                    out=oall[:].rearrange("p (k s) -> p k s", s=N_SP),
                    in0=mask_f[:].to_broadcast([P, K, N_SP]),
                    in1=iota_tile[:].rearrange("p (k s) -> p k s", s=N_SP),
                    op=mybir.AluOpType.is_equal,
                )
                oalls.append(oall)
            return oalls

        oalls = None if dma_only else emit_prologue()

        for r in range(reps):
            if compute_only:
                chunks = static_chunks
            else:
                chunks = []
                for cj in range(nchunk):
                    ct = feats_pool.tile([P, KPC * T * C], fdt)
                    eng = nc.sync if cj % 2 == 0 else nc.scalar
                    eng.dma_start(
                        out=ct[:],
                        in_=maps_t.ap()[:, cj * KPC * T * C : (cj + 1) * KPC * T * C],
                    )
                    chunks.append(ct)

            if dma_only:
                probe = small_pool.tile([P, 1], f32)
                nc.vector.reduce_sum(
                    out=probe[:], in_=chunks[-1][:, :4], axis=mybir.AxisListType.X
                )
                continue

            cur_oalls = oalls
            if r + 1 < reps:
                oalls = emit_prologue()  # next rep's, built during this rep

            # stage A: fat matmuls, one-hot stationary, feats streamed N=512
            segcs = [
                segc_pool.tile([N_SP, C], f32, name=f"segc{t}")
                for t in range(T)
            ]
            for cj in range(nchunk):
                ct = chunks[cj]
                for t in range(T):
                    for kk in range(KPC):
                        k = cj * KPC + kk
                        nc.tensor.matmul(
                            out=segcs[t][:],
                            lhsT=cur_oalls[t][:, k * N_SP : (k + 1) * N_SP],
                            rhs=ct[:, kk * T * C + t * C : kk * T * C + (t + 1) * C],
                            start=(k == 0),
                            stop=(k == K - 1),
                        )

            # stage B (phase-major): PSUM->SBUF, 4 PE transposes per frame,
            # then project through W^T
            segsbs, ctsbs, segs, segfs_l = [], [], [], []
            for t in range(T):
                segc_sb = segsb_pool.tile([N_SP, C], bf16)
                copy(out=segc_sb[:], in_=segcs[t][:])
                segsbs.append(segc_sb)
            for t in range(T):
                ct_sb = ctsb_pool.tile([P, CB * N_SP], bf16)
                for cb in range(CB):
                    ctp = tp_pool.tile([P, N_SP], bf16, name="tp")
                    nc.tensor.transpose(
                        out=ctp[:],
                        in_=segsbs[t][:, cb * P : (cb + 1) * P],
                        identity=identity[:N_SP, :N_SP],
                    )
                    copy(out=ct_sb[:, cb * N_SP : (cb + 1) * N_SP], in_=ctp[:])
                ctsbs.append(ct_sb)
            for t in range(T):
                seg = seg_pool.tile([N_SP, D_OUT], f32)
                for cb in range(CB):
                    nc.tensor.matmul(
                        out=seg[:],
                        lhsT=ctsbs[t][:, cb * N_SP : (cb + 1) * N_SP],
                        rhs=wt_tile[:, cb * D_OUT : (cb + 1) * D_OUT],
                        start=(cb == 0),
                        stop=(cb == CB - 1),
                    )
                segs.append(seg)
            outsb = outsb_pool.tile([N_SP, T * D_OUT], f32)
            for t in range(T):
                seg_fs = segfs_pool.tile([N_SP, D_OUT], f32)
                nc.vector.tensor_copy(out=seg_fs[:], in_=segs[t][:])
                segfs_l.append(seg_fs)
                sq = small_pool.tile([N_SP, D_OUT], f32)
                ss = small_pool.tile([N_SP, 1], f32)
                nc.vector.tensor_mul(out=sq[:], in0=seg_fs[:], in1=seg_fs[:])
                nc.vector.reduce_sum(
                    out=ss[:], in_=sq[:], axis=mybir.AxisListType.X
                )
                nrm = small_pool.tile([N_SP, 1], f32)
                nc.scalar.activation(
                    out=nrm[:],
                    in_=ss[:],
                    func=mybir.ActivationFunctionType.Sqrt,
                    bias=eps_tile[:N_SP],
                )
                inv = small_pool.tile([N_SP, 1], f32)
                nc.vector.reciprocal(out=inv[:], in_=nrm[:])
                nc.vector.tensor_scalar_mul(
                    out=outsb[:, t * D_OUT : (t + 1) * D_OUT],
                    in0=segfs_l[t][:],
                    scalar1=inv[:],
                )
            nc.gpsimd.dma_start(
                out=out_t.ap().rearrange("t s d -> s t d"),
                in_=outsb[:].rearrange("s (t d) -> s t d", d=D_OUT),
            )

    nc.compile()
    return nc


VARIANT = "pkbf"


def _get_program():
    if "nc" not in _CACHE:
        if VARIANT == "pm":
            _CACHE["nc"] = _build_program_pm()
        elif VARIANT == "pkb":
            _CACHE["nc"] = _build_program_pkb()
        elif VARIANT == "pkb8":
            _CACHE["nc"] = _build_program_pkb(fp8=True)
        elif VARIANT == "pkbf":
            _CACHE["nc"] = _build_program_pkbf()
        else:
            _CACHE["nc"] = _build_program()
    return _CACHE["nc"]


def _to_pkb(maps_q):
    """(B,C,T,H,W) -> (B, P, K*T*C) with column layout (k, t, c)."""
    return np.ascontiguousarray(
        maps_q.reshape(B, C, T, HW // P, P)
        .transpose(0, 4, 3, 2, 1)
        .reshape(B, P, (HW // P) * T * C)
    )


def _prep_in_maps(maps, sp_mask, W_fc):
    """Host-side input prep (dtype cast + layout) for the current VARIANT."""
    import ml_dtypes

    bf16 = ml_dtypes.bfloat16
    maps = np.asarray(maps, dtype=np.float32)
    sp_mask = np.asarray(sp_mask, dtype=np.int32)
    W_fc = np.asarray(W_fc, dtype=np.float32)
    assert maps.shape == (B, C, T, H, W)
    wt = np.ascontiguousarray(W_fc.T).astype(bf16)  # (C, D_OUT)
    if VARIANT == "pm":
        maps_pm = np.ascontiguousarray(
            maps.astype(bf16).transpose(0, 2, 3, 4, 1).reshape(B, T, HW, C)
        )
        return [
            {"maps_pm": maps_pm[b], "sp_mask": sp_mask[b], "W_fcT": wt}
            for b in range(B)
        ]
    if VARIANT in ("pkb", "pkb8", "pkbf"):
        fdt = bf16 if VARIANT == "pkb" else ml_dtypes.float8_e3m4
        maps_pkb = _to_pkb(maps.astype(fdt))
        return [
            {"maps_pkb": maps_pkb[b], "sp_mask": sp_mask[b], "W_fcT": wt}
            for b in range(B)
        ]
    return [
        {"maps_bf16": maps[b].astype(bf16), "sp_mask": sp_mask[b], "W_fcT": wt}
        for b in range(B)
    ]


def kernel(maps, sp_mask, W_fc, max_sp_num):
    from concourse.bass_utils import run_bass_kernel_spmd

    assert int(max_sp_num) == N_SP
    in_maps = _prep_in_maps(maps, sp_mask, W_fc)
    nc = _get_program()
    res = run_bass_kernel_spmd(nc, in_maps, core_ids=list(range(N_CORES)))
    # per-core out is (T, N_SP, D_OUT); full output is (B, D_OUT, T, N_SP)
    out = np.stack([res.results[b]["out"] for b in range(B)], axis=0)
    return np.ascontiguousarray(out.transpose(0, 3, 1, 2)).astype(np.float32)



# revision 27
# speedup vs baseline: 1.8129x; 1.8129x over previous
"""Trainium2 Bass kernel for segment-mean + linear head + L2-normalize.

Reference computation (per batch element b, frame t):
  mean[s, c]  = mean over pixels p with sp_mask[p] == s of maps[c, p]
  sp[s, d]    = sum_c mean[s, c] * W_fc[d, c]
  out[d, s]   = sp[s, d] / max(||sp[s, :]||_2, 1e-12)

Key identities/choices:
- the per-segment count division cancels inside the L2 normalization
  (normalize(v / n) == normalize(v) for n > 0, and empty segments are
  exactly zero either way), so counts are never computed.
- maps are pre-cast to float8_e3m4 on the host (4 mantissa bits): the
  kernel is HBM-bound, fp8 halves read volume vs bf16. Measured end-to-end
  rel err 1.40e-2 (vs 3.0e-3 at bf16) on the seed-0 inputs.
- data-parallel over B: one clip per NeuronCore, 8 cores.

Active variant "pkbf" per clip (see _build_program_pkbf):
  1. host relayouts maps to (P, K*T*C) fp8, pixel index = k*128 + p; the
     clip is DMAed in 8 x 1MB chunks alternating the two HWDGE rings.
  2. one-hot O[p, (k,s)] = (sp_mask==s) built on DVE (iota + is_equal),
     pipelined one rep ahead.
  3. stage A (PE): segc[s, c] += O_kt[p, s]^T @ feats[p, (k,t,:)] -- the
     one-hot is the STATIONARY operand, feats stream as the moving operand
     (N=512). 128 fat matmuls/clip. This matters: LDWEIGHTS runs at
     ~1 col/1.2GHz-cycle (no FWL on this toolchain), so thin matmuls
     (feats stationary, N=100) are LDWEIGHTS-bound at ~107ns -> 55us/clip,
     while fat ones hide the 83ns weight load under the 213ns stream and
     approach the 65.5k-cycle (27us) feats-through-PE floor.
  4. stage B (PE): 4 transposes of segc per frame + project through W^T,
     PSUM-bank layout: 4 segc banks + 2 seg + 2 transpose staging = 8.
  5. norm epilogue on DVE (+4 ACT sqrts), one gpsimd output DMA per clip.

Measured (8 cores in parallel, marginal two-slope method, clean window):
  full ~27.8-29.0us/clip, compute-only ~34.8us, fp8 DMA floor ~21.5us
  (vs 87.4us graded baseline at bf16 with thin matmuls). oall_bufs=8 is
  load-bearing: with 6, next-rep one-hot builds wait on this rep's whole
  stage A and the boundary stall serializes DMA behind compute (~3x).
"""

import numpy as np

B, C, T, H, W = 8, 512, 4, 64, 64
HW = H * W          # 4096 pixels per frame
N_SP = 100
D_OUT = 128
N_CORES = 8
P = 128             # SBUF partitions
NCH = C // P        # 4 channel chunks
NPIX = HW // P      # 32 pixel chunks

_CACHE = {}


def _build_program(reps=1, feats_bufs=3, dma_split=2, dma_only=False, pp_bufs=3, proj_bufs=4, compute_only=False):
    from contextlib import ExitStack

    import concourse.tile as tile
    from concourse import bacc, mybir
    from concourse.masks import make_identity

    f32 = mybir.dt.float32
    bf16 = mybir.dt.bfloat16
    i32 = mybir.dt.int32

    nc = bacc.Bacc(
        "TRN2",
        target_bir_lowering=False,
        debug=False,
        num_devices=N_CORES,
    )

    # maps and W arrive pre-cast to bf16 (host-side) -- the kernel computes in
    # bf16 anyway, so this halves the HBM read volume at identical accuracy
    maps_t = nc.dram_tensor("maps_bf16", [C, T, H, W], bf16, kind="ExternalInput")
    mask_t = nc.dram_tensor("sp_mask", [T, H, W], i32, kind="ExternalInput")
    wfc_t = nc.dram_tensor("W_fcT", [C, D_OUT], bf16, kind="ExternalInput")
    out_t = nc.dram_tensor("out", [T, N_SP, D_OUT], f32, kind="ExternalOutput")

    # DRAM views
    # maps as (c_in_chunk, chunk, t, pixel): partition dim = channel-in-chunk
    maps_r = maps_t.ap().rearrange("(cj c) t h w -> c cj t (h w)", c=P)
    # sp_mask frame as (chunk, pixel_in_chunk): chunk k = pixels [128k, 128k+128)
    mask_r = mask_t.ap().rearrange("t h w -> t (h w)").rearrange(
        "t (k p) -> t k p", p=P
    )
    # W_fc^T per channel chunk: (c_in_chunk, chunk, d)
    wfc_r = wfc_t.ap().rearrange("(cj c) d -> c cj d", c=P)

    with tile.TileContext(nc) as tc, ExitStack() as ctx:
        const_pool = ctx.enter_context(tc.tile_pool(name="const", bufs=1))
        feats_pool = ctx.enter_context(tc.tile_pool(name="feats", bufs=feats_bufs))
        mask_pool = ctx.enter_context(tc.tile_pool(name="mask", bufs=4))
        maskf_pool = ctx.enter_context(tc.tile_pool(name="maskf", bufs=4))
        oall_pool = ctx.enter_context(tc.tile_pool(name="oall", bufs=4))
        proj_pool = ctx.enter_context(tc.tile_pool(name="proj", bufs=proj_bufs))
        outsb_pool = ctx.enter_context(tc.tile_pool(name="outsb", bufs=2))
        small_pool = ctx.enter_context(tc.tile_pool(name="small", bufs=4))
        pp_pool = ctx.enter_context(tc.tile_pool(name="pp", bufs=pp_bufs, space="PSUM"))
        seg_pool = ctx.enter_context(tc.tile_pool(name="seg", bufs=2, space="PSUM"))
        mtp_pool = ctx.enter_context(tc.tile_pool(name="mtp", bufs=2, space="PSUM"))

        # iota constant: column k*100+s holds value s (segment id pattern).
        # Values are 0..99, exact in f32.
        iota_tile = const_pool.tile([P, NPIX * N_SP], f32)
        nc.gpsimd.iota(
            iota_tile[:].rearrange("p (k s) -> p k s", s=N_SP),
            pattern=[[0, NPIX], [1, N_SP]],
            base=0,
            channel_multiplier=0,
            allow_small_or_imprecise_dtypes=True,
        )

        identity = const_pool.tile([P, P], f32)
        make_identity(nc, identity[:])

        # tiny positive bias so sqrt(ss + eps) never hits 1/0 on empty segments
        eps_tile = const_pool.tile([P, 1], f32)
        nc.vector.memset(eps_tile[:], 1e-30)

        # W^T in SBUF as bf16: column block cj holds (c_in_chunk, d) for chunk cj
        wt_tile = const_pool.tile([P, NCH * D_OUT], bf16)
        nc.sync.dma_start(
            out=wt_tile[:].rearrange("c (cj d) -> c cj d", d=D_OUT),
            in_=wfc_r,
        )

        static_feats = None
        if compute_only:
            static_feats = const_pool.tile([P, NCH * HW], bf16)
            nc.sync.dma_start(
                out=static_feats[:].rearrange("c (cj p) -> c cj p", p=HW),
                in_=maps_r[:, :, 0, :],
            )

        # prologue: build all four frames' one-hot matrices up front -- they
        # depend only on the tiny sp_mask tensor, so this runs entirely under
        # the first feats DMA instead of on each frame's critical path
        oalls = []
        if not dma_only:
            for t in range(T):
                mask_raw = mask_pool.tile([NPIX, P], f32)
                nc.gpsimd.dma_start(out=mask_raw[:], in_=mask_r[t])
                mask_ps = mtp_pool.tile([P, NPIX], f32)
                nc.tensor.transpose(
                    out=mask_ps[:], in_=mask_raw[:], identity=identity[:NPIX, :NPIX]
                )
                mask_f = maskf_pool.tile([P, NPIX], f32)
                nc.scalar.copy(out=mask_f[:], in_=mask_ps[:])
                oall = oall_pool.tile([P, NPIX * N_SP], bf16)
                nc.vector.tensor_tensor(
                    out=oall[:].rearrange("p (k s) -> p k s", s=N_SP),
                    in0=mask_f[:].to_broadcast([P, NPIX, N_SP]),
                    in1=iota_tile[:].rearrange("p (k s) -> p k s", s=N_SP),
                    op=mybir.AluOpType.is_equal,
                )
                oalls.append(oall)

        for t in [t for _ in range(reps) for t in range(T)]:
            if compute_only:
                feats = static_feats
            else:
                feats = feats_pool.tile([P, NCH * HW], bf16)
            for sp_i in range(0 if compute_only else dma_split):
                lo = sp_i * (NCH // dma_split)
                hi = (sp_i + 1) * (NCH // dma_split)
                dma_eng = nc.scalar if (sp_i % 2 == 1) else nc.sync
                dma_eng.dma_start(
                    out=feats[:, lo * HW : hi * HW].rearrange(
                        "c (cj p) -> c cj p", p=HW
                    ),
                    in_=maps_r[:, lo:hi, t, :],
                )

            if dma_only:
                # measure the pure input-DMA floor: touch feats with one tiny
                # op per frame so the loads stay live, skip all compute
                probe = small_pool.tile([P, 1], f32)
                nc.vector.reduce_sum(
                    out=probe[:], in_=feats[:, :4], axis=mybir.AxisListType.X
                )
                continue

            oall = oalls[t]
            seg = seg_pool.tile([N_SP, D_OUT], f32)
            for g in range(NPIX // 4):
                pp = pp_pool.tile([P, 4 * D_OUT], f32)
                for i in range(4):
                    ch = g * 4 + i
                    for cj in range(NCH):
                        nc.tensor.matmul(
                            out=pp[:, i * D_OUT : (i + 1) * D_OUT],
                            lhsT=feats[:, cj * HW + ch * P : cj * HW + (ch + 1) * P],
                            rhs=wt_tile[:, cj * D_OUT : (cj + 1) * D_OUT],
                            start=(cj == 0),
                            stop=(cj == NCH - 1),
                        )
                proj = proj_pool.tile([P, 4 * D_OUT], bf16)
                nc.scalar.copy(out=proj[:], in_=pp[:])
                for i in range(4):
                    ch = g * 4 + i
                    nc.tensor.matmul(
                        out=seg[:],
                        lhsT=oall[:, ch * N_SP : (ch + 1) * N_SP],
                        rhs=proj[:, i * D_OUT : (i + 1) * D_OUT],
                        start=(ch == 0),
                        stop=(ch == NPIX - 1),
                    )

            sq = small_pool.tile([N_SP, D_OUT], f32)
            ss = small_pool.tile([N_SP, 1], f32)
            nc.scalar.activation(
                out=sq[:],
                in_=seg[:],
                func=mybir.ActivationFunctionType.Square,
                accum_out=ss[:],
            )
            nrm = small_pool.tile([N_SP, 1], f32)
            nc.scalar.activation(
                out=nrm[:],
                in_=ss[:],
                func=mybir.ActivationFunctionType.Sqrt,
                bias=eps_tile[:N_SP],
            )
            inv = small_pool.tile([N_SP, 1], f32)
            nc.vector.reciprocal(out=inv[:], in_=nrm[:])
            outsb = outsb_pool.tile([N_SP, D_OUT], f32)
            nc.vector.tensor_scalar_mul(out=outsb[:], in0=seg[:], scalar1=inv[:])
            nc.sync.dma_start(out=out_t.ap()[t], in_=outsb[:])

    nc.compile()
    return nc


def _build_program_pm(reps=1, feats_bufs=3, dma_split=2, dma_only=False):
    """Pixel-major variant: host supplies maps as (T, HW, C) bf16.

    Per frame: one-hot segment-sum over raw features first
    (seg_c[s, c] = sum_p O[p, s] * feats[p, c], 32 matmuls of N=512),
    then project the tiny (100, 512) result through W^T (4 matmuls of
    N=128 after 4 PE transposes). ~210M MACs/frame vs 320M for the
    projection-first ordering, and no big PSUM->SBUF copy traffic.
    """
    from contextlib import ExitStack

    import concourse.tile as tile
    from concourse import bacc, mybir
    from concourse.masks import make_identity

    f32 = mybir.dt.float32
    bf16 = mybir.dt.bfloat16
    i32 = mybir.dt.int32

    nc = bacc.Bacc(
        "TRN2",
        target_bir_lowering=False,
        debug=False,
        num_devices=N_CORES,
    )

    maps_t = nc.dram_tensor("maps_pm", [T, HW, C], bf16, kind="ExternalInput")
    mask_t = nc.dram_tensor("sp_mask", [T, H, W], i32, kind="ExternalInput")
    wfc_t = nc.dram_tensor("W_fcT", [C, D_OUT], bf16, kind="ExternalInput")
    out_t = nc.dram_tensor("out", [T, N_SP, D_OUT], f32, kind="ExternalOutput")

    # (pixel_in_chunk, chunk, t, channel)
    maps_r = maps_t.ap().rearrange("t (k p) c -> p k t c", p=P)
    mask_r = mask_t.ap().rearrange("t h w -> t (h w)").rearrange(
        "t (k p) -> t k p", p=P
    )
    wfc_r = wfc_t.ap().rearrange("(cj c) d -> c cj d", c=P)

    with tile.TileContext(nc) as tc, ExitStack() as ctx:
        const_pool = ctx.enter_context(tc.tile_pool(name="const", bufs=1))
        feats_pool = ctx.enter_context(tc.tile_pool(name="feats", bufs=feats_bufs))
        mask_pool = ctx.enter_context(tc.tile_pool(name="mask", bufs=4))
        maskf_pool = ctx.enter_context(tc.tile_pool(name="maskf", bufs=4))
        oall_pool = ctx.enter_context(tc.tile_pool(name="oall", bufs=4))
        segsb_pool = ctx.enter_context(tc.tile_pool(name="segsb", bufs=2))
        ctsb_pool = ctx.enter_context(tc.tile_pool(name="ctsb", bufs=2))
        outsb_pool = ctx.enter_context(tc.tile_pool(name="outsb", bufs=2))
        small_pool = ctx.enter_context(tc.tile_pool(name="small", bufs=4))
        segc_pool = ctx.enter_context(tc.tile_pool(name="segc", bufs=2, space="PSUM"))
        ct_pool = ctx.enter_context(tc.tile_pool(name="ct", bufs=2, space="PSUM"))
        seg_pool = ctx.enter_context(tc.tile_pool(name="seg", bufs=2, space="PSUM"))
        mtp_pool = ctx.enter_context(tc.tile_pool(name="mtp", bufs=2, space="PSUM"))

        # iota constant (bf16: values 0..99 are exact)
        iota_tile = const_pool.tile([P, NPIX * N_SP], bf16)
        nc.gpsimd.iota(
            iota_tile[:].rearrange("p (k s) -> p k s", s=N_SP),
            pattern=[[0, NPIX], [1, N_SP]],
            base=0,
            channel_multiplier=0,
            allow_small_or_imprecise_dtypes=True,
        )

        identity = const_pool.tile([P, P], bf16)
        make_identity(nc, identity[:])

        eps_tile = const_pool.tile([P, 1], f32)
        nc.vector.memset(eps_tile[:], 1e-30)

        wt_tile = const_pool.tile([P, NCH * D_OUT], bf16)
        nc.sync.dma_start(
            out=wt_tile[:].rearrange("c (cj d) -> c cj d", d=D_OUT),
            in_=wfc_r,
        )

        for t in [t for _ in range(reps) for t in range(T)]:
            feats = feats_pool.tile([P, NPIX * C], bf16)
            for sp_i in range(dma_split):
                lo = sp_i * (NPIX // dma_split)
                hi = (sp_i + 1) * (NPIX // dma_split)
                dma_eng = nc.scalar if (sp_i % 2 == 1) else nc.sync
                dma_eng.dma_start(
                    out=feats[:, lo * C : hi * C].rearrange(
                        "p (k c) -> p k c", c=C
                    ),
                    in_=maps_r[:, lo:hi, t, :],
                )

            if dma_only:
                probe = small_pool.tile([P, 1], f32)
                nc.vector.reduce_sum(
                    out=probe[:], in_=feats[:, :4], axis=mybir.AxisListType.X
                )
                continue

            # mask: (chunk, pixel) i32 -> bf16, PE-transpose to (pixel, chunk)
            mask_raw = mask_pool.tile([NPIX, P], bf16)
            nc.gpsimd.dma_start(out=mask_raw[:], in_=mask_r[t])
            mask_ps = mtp_pool.tile([P, NPIX], bf16)
            nc.tensor.transpose(
                out=mask_ps[:], in_=mask_raw[:], identity=identity[:NPIX, :NPIX]
            )
            mask_f = maskf_pool.tile([P, NPIX], bf16)
            nc.scalar.copy(out=mask_f[:], in_=mask_ps[:])

            oall = oall_pool.tile([P, NPIX * N_SP], bf16)
            nc.vector.tensor_tensor(
                out=oall[:].rearrange("p (k s) -> p k s", s=N_SP),
                in0=mask_f[:].to_broadcast([P, NPIX, N_SP]),
                in1=iota_tile[:].rearrange("p (k s) -> p k s", s=N_SP),
                op=mybir.AluOpType.is_equal,
            )

            # stage A: per-segment channel sums, seg_c (100 s, 512 c)
            segc = segc_pool.tile([N_SP, C], f32)
            for k in range(NPIX):
                nc.tensor.matmul(
                    out=segc[:],
                    lhsT=oall[:, k * N_SP : (k + 1) * N_SP],
                    rhs=feats[:, k * C : (k + 1) * C],
                    start=(k == 0),
                    stop=(k == NPIX - 1),
                )
            segc_sb = segsb_pool.tile([N_SP, C], bf16)
            nc.scalar.copy(out=segc_sb[:], in_=segc[:])

            # transpose seg_c -> (c, s) per channel chunk, then project
            ct_sb = ctsb_pool.tile([P, NCH * N_SP], bf16)
            for cj in range(NCH):
                ctp = ct_pool.tile([P, N_SP], bf16)
                nc.tensor.transpose(
                    out=ctp[:],
                    in_=segc_sb[:, cj * P : (cj + 1) * P],
                    identity=identity[:N_SP, :N_SP],
                )
                nc.scalar.copy(
                    out=ct_sb[:, cj * N_SP : (cj + 1) * N_SP], in_=ctp[:]
                )

            # stage B: seg (100 s, 128 d) = seg_c @ W^T
            seg = seg_pool.tile([N_SP, D_OUT], f32)
            for cj in range(NCH):
                nc.tensor.matmul(
                    out=seg[:],
                    lhsT=ct_sb[:, cj * N_SP : (cj + 1) * N_SP],
                    rhs=wt_tile[:, cj * D_OUT : (cj + 1) * D_OUT],
                    start=(cj == 0),
                    stop=(cj == NCH - 1),
                )

            sq = small_pool.tile([N_SP, D_OUT], f32)
            ss = small_pool.tile([N_SP, 1], f32)
            nc.scalar.activation(
                out=sq[:],
                in_=seg[:],
                func=mybir.ActivationFunctionType.Square,
                accum_out=ss[:],
            )
            nrm = small_pool.tile([N_SP, 1], f32)
            nc.scalar.activation(
                out=nrm[:],
                in_=ss[:],
                func=mybir.ActivationFunctionType.Sqrt,
                bias=eps_tile[:N_SP],
            )
            inv = small_pool.tile([N_SP, 1], f32)
            nc.vector.reciprocal(out=inv[:], in_=nrm[:])
            outsb = outsb_pool.tile([N_SP, D_OUT], f32)
            nc.vector.tensor_scalar_mul(out=outsb[:], in0=seg[:], scalar1=inv[:])
            nc.sync.dma_start(out=out_t.ap()[t], in_=outsb[:])

    nc.compile()
    return nc


def _build_program_pkb(reps=1, dma_only=False, compute_only=False, nchunk=8,
                       fp8=False, feats_bufs=None, oall_bufs=6):
    """Pixel-blocked variant: host supplies maps as (P, K*T*C) bf16 (or
    float8_e3m4 when fp8=True), where
    column layout is (k, t, c) — pixel index = k*128 + p.

    Design (see measurements in session notes):
    - segment-sum FIRST on raw features with feats-stationary matmuls
      (contraction over pixels on the partition dim): 51.2k PE cycles/clip
      vs 82k for projection-first.
        stage A: segcT[c,s] += feats[p,(k,t,cb)]^T @ onehot[p,(k,s)]
        stage B: seg[s,d] = segcT^T @ W^T per channel block
    - k-major DMA layout gives long contiguous DRAM runs; the clip is split
      into `nchunk` chunk-tiles (16KB runs at nchunk=8) alternating the two
      HWDGE rings, and stage A is chunk-major so compute consumes each chunk
      as it lands and next-rep DMA overlaps this rep's tail compute.
    - ONE matmul `start` per PSUM bank (start marks the whole 2KB zero
      region pending-zero; other cb regions self-initialize on first write).
    - mask -> one-hot prologue for rep r+1 is emitted during rep r, so the
      PE/DVE never stall on it at rep boundaries.
    - epilogue runs on DVE (copies, square+reduce, reciprocal, scale); the
      ACT queue carries only its HWDGE ring triggers + 4 tiny sqrts, so
      next-rep input DMA triggers are not stuck behind epilogue compute.
    - all four frame outputs leave in ONE gpsimd DMA per clip (small per-rep
      DMAs on the input rings were measured to stall them).
    """
    from contextlib import ExitStack

    import concourse.tile as tile
    from concourse import bacc, mybir
    from concourse.masks import make_identity

    f32 = mybir.dt.float32
    bf16 = mybir.dt.bfloat16
    i32 = mybir.dt.int32
    fdt = mybir.dt.float8e3 if fp8 else bf16

    K = NPIX            # 32 k-chunks of 128 pixels
    KTC = K * T * C     # 65536 columns per partition
    CB = 4              # channel blocks of 128
    KPC = K // nchunk   # k-chunks per DMA chunk
    if feats_bufs is None:
        feats_bufs = nchunk

    nc = bacc.Bacc(
        "TRN2",
        target_bir_lowering=False,
        debug=False,
        num_devices=N_CORES,
    )

    maps_t = nc.dram_tensor("maps_pkb", [P, KTC], fdt, kind="ExternalInput")
    mask_t = nc.dram_tensor("sp_mask", [T, H, W], i32, kind="ExternalInput")
    wfc_t = nc.dram_tensor("W_fcT", [C, D_OUT], bf16, kind="ExternalInput")
    out_t = nc.dram_tensor("out", [T, N_SP, D_OUT], f32, kind="ExternalOutput")

    mask_r = mask_t.ap().rearrange("t h w -> t (h w)").rearrange(
        "t (k p) -> t k p", p=P
    )
    wfc_r = wfc_t.ap().rearrange("(cj c) d -> c cj d", c=P)

    with tile.TileContext(nc) as tc, ExitStack() as ctx:
        const_pool = ctx.enter_context(tc.tile_pool(name="const", bufs=1))
        feats_pool = ctx.enter_context(tc.tile_pool(name="feats", bufs=feats_bufs))
        mask_pool = ctx.enter_context(tc.tile_pool(name="mask", bufs=8))
        maskf_pool = ctx.enter_context(tc.tile_pool(name="maskf", bufs=8))
        oall_pool = ctx.enter_context(tc.tile_pool(name="oall", bufs=oall_bufs))
        segsb_pool = ctx.enter_context(tc.tile_pool(name="segsb", bufs=4))
        segfs_pool = ctx.enter_context(tc.tile_pool(name="segfs", bufs=4))
        outsb_pool = ctx.enter_context(tc.tile_pool(name="outsb", bufs=2))
        small_pool = ctx.enter_context(tc.tile_pool(name="small", bufs=4))
        segct_pool = ctx.enter_context(
            tc.tile_pool(name="segct", bufs=1, space="PSUM")
        )
        seg_pool = ctx.enter_context(tc.tile_pool(name="seg", bufs=2, space="PSUM"))
        mtp_pool = ctx.enter_context(tc.tile_pool(name="mtp", bufs=2, space="PSUM"))

        # iota constant: column (k, s) holds value s (bf16: 0..99 exact)
        iota_tile = const_pool.tile([P, K * N_SP], bf16)
        nc.gpsimd.iota(
            iota_tile[:].rearrange("p (k s) -> p k s", s=N_SP),
            pattern=[[0, K], [1, N_SP]],
            base=0,
            channel_multiplier=0,
            allow_small_or_imprecise_dtypes=True,
        )

        identity = const_pool.tile([P, P], bf16)
        make_identity(nc, identity[:])

        eps_tile = const_pool.tile([P, 1], f32)
        nc.vector.memset(eps_tile[:], 1e-30)

        wt_tile = const_pool.tile([P, CB * D_OUT], bf16)
        nc.sync.dma_start(
            out=wt_tile[:].rearrange("c (cj d) -> c cj d", d=D_OUT),
            in_=wfc_r,
        )

        static_chunks = None
        if compute_only:
            static_chunks = []
            for cj in range(nchunk):
                sc = const_pool.tile([P, KPC * T * C], fdt, name=f"static{cj}")
                nc.sync.dma_start(
                    out=sc[:],
                    in_=maps_t.ap()[:, cj * KPC * T * C : (cj + 1) * KPC * T * C],
                )
                static_chunks.append(sc)

        def emit_prologue():
            """mask -> transpose -> one-hot for one rep; returns oall tiles."""
            oalls = []
            for t in range(T):
                mask_raw = mask_pool.tile([K, P], bf16)
                nc.gpsimd.dma_start(out=mask_raw[:], in_=mask_r[t])
                mask_ps = mtp_pool.tile([P, K], bf16)
                nc.tensor.transpose(
                    out=mask_ps[:], in_=mask_raw[:], identity=identity[:K, :K]
                )
                mask_f = maskf_pool.tile([P, K], bf16)
                nc.vector.tensor_copy(out=mask_f[:], in_=mask_ps[:])
                oall = oall_pool.tile([P, K * N_SP], fdt)
                nc.vector.tensor_tensor(
                    out=oall[:].rearrange("p (k s) -> p k s", s=N_SP),
                    in0=mask_f[:].to_broadcast([P, K, N_SP]),
                    in1=iota_tile[:].rearrange("p (k s) -> p k s", s=N_SP),
                    op=mybir.AluOpType.is_equal,
                )
                oalls.append(oall)
            return oalls

        oalls = None if dma_only else emit_prologue()

        for r in range(reps):
            if compute_only:
                chunks = static_chunks
            else:
                chunks = []
                for cj in range(nchunk):
                    ct = feats_pool.tile([P, KPC * T * C], fdt)
                    eng = nc.sync if cj % 2 == 0 else nc.scalar
                    eng.dma_start(
                        out=ct[:],
                        in_=maps_t.ap()[:, cj * KPC * T * C : (cj + 1) * KPC * T * C],
                    )
                    chunks.append(ct)

            if dma_only:
                probe = small_pool.tile([P, 1], f32)
                nc.vector.reduce_sum(
                    out=probe[:], in_=chunks[-1][:, :4], axis=mybir.AxisListType.X
                )
                continue

            cur_oalls = oalls
            if r + 1 < reps:
                oalls = emit_prologue()  # next rep's, built during this rep

            # stage A: chunk-major; ONE start/stop per frame PSUM bank
            segcts = [
                segct_pool.tile([P, CB * N_SP], f32, name=f"segct{t}")
                for t in range(T)
            ]
            for cj in range(nchunk):
                ct = chunks[cj]
                for t in range(T):
                    for kk in range(KPC):
                        k = cj * KPC + kk
                        for cb in range(CB):
                            nc.tensor.matmul(
                                out=segcts[t][:, cb * N_SP : (cb + 1) * N_SP],
                                lhsT=ct[
                                    :,
                                    kk * T * C + t * C + cb * P : kk * T * C
                                    + t * C
                                    + (cb + 1) * P,
                                ],
                                rhs=cur_oalls[t][:, k * N_SP : (k + 1) * N_SP],
                                start=(k == 0 and cb == 0),
                                stop=(k == K - 1 and cb == CB - 1),
                            )

            # stage B (phase-major so engines overlap across frames)
            segsbs, segs, segfs_l = [], [], []
            for t in range(T):
                segct_sb = segsb_pool.tile([P, CB * N_SP], bf16)
                nc.vector.tensor_copy(out=segct_sb[:], in_=segcts[t][:])
                segsbs.append(segct_sb)
            for t in range(T):
                seg = seg_pool.tile([N_SP, D_OUT], f32)
                for cb in range(CB):
                    nc.tensor.matmul(
                        out=seg[:],
                        lhsT=segsbs[t][:, cb * N_SP : (cb + 1) * N_SP],
                        rhs=wt_tile[:, cb * D_OUT : (cb + 1) * D_OUT],
                        start=(cb == 0),
                        stop=(cb == CB - 1),
                    )
                segs.append(seg)
            outsb = outsb_pool.tile([N_SP, T * D_OUT], f32)
            for t in range(T):
                # seg -> SBUF copy frees the PSUM bank after two DVE reads
                seg_fs = segfs_pool.tile([N_SP, D_OUT], f32)
                nc.vector.tensor_copy(out=seg_fs[:], in_=segs[t][:])
                segfs_l.append(seg_fs)
                sq = small_pool.tile([N_SP, D_OUT], f32)
                ss = small_pool.tile([N_SP, 1], f32)
                nc.vector.tensor_mul(out=sq[:], in0=seg_fs[:], in1=seg_fs[:])
                nc.vector.reduce_sum(
                    out=ss[:], in_=sq[:], axis=mybir.AxisListType.X
                )
                nrm = small_pool.tile([N_SP, 1], f32)
                nc.scalar.activation(
                    out=nrm[:],
                    in_=ss[:],
                    func=mybir.ActivationFunctionType.Sqrt,
                    bias=eps_tile[:N_SP],
                )
                inv = small_pool.tile([N_SP, 1], f32)
                nc.vector.reciprocal(out=inv[:], in_=nrm[:])
                nc.vector.tensor_scalar_mul(
                    out=outsb[:, t * D_OUT : (t + 1) * D_OUT],
                    in0=segfs_l[t][:],
                    scalar1=inv[:],
                )
            nc.gpsimd.dma_start(
                out=out_t.ap().rearrange("t s d -> s t d"),
                in_=outsb[:].rearrange("s (t d) -> s t d", d=D_OUT),
            )

    nc.compile()
    return nc


def _build_program_pkbf(reps=1, dma_only=False, compute_only=False, nchunk=8,
                        feats_bufs=None, oall_bufs=8, copy_eng="vector"):
    """Fat-matmul fp8 variant: host supplies maps as (P, K*T*C) float8_e3m4,
    column layout (k, t, c) — pixel index = k*128 + p.

    Stage A makes the ONE-HOT the stationary operand and streams feats as
    the moving operand:
        segc[s, c] += onehot_kt[p, s]^T @ feats[p, (k,t,:)]     (N=512)
    128 matmuls/clip instead of 512. On this hardware LDWEIGHTS runs at
    ~1 col / 1.2GHz-cycle (no FWL) and binds thin matmuls at ~107ns; with
    the fat stream the 83ns one-hot load hides under the 213ns stream, so
    stage A approaches the 65.5k-cycle (27us) stream floor.

    Stage B transposes segc per channel block on the PE (16 small
    transposes) and projects through W^T as in the pm variant.
    """
    from contextlib import ExitStack

    import concourse.tile as tile
    from concourse import bacc, mybir
    from concourse.masks import make_identity

    f32 = mybir.dt.float32
    bf16 = mybir.dt.bfloat16
    i32 = mybir.dt.int32
    fdt = mybir.dt.float8e3

    K = NPIX            # 32 k-chunks of 128 pixels
    KTC = K * T * C     # 65536 columns per partition
    CB = 4              # channel blocks of 128
    KPC = K // nchunk   # k-chunks per DMA chunk
    if feats_bufs is None:
        feats_bufs = nchunk

    nc = bacc.Bacc(
        "TRN2",
        target_bir_lowering=False,
        debug=False,
        num_devices=N_CORES,
    )

    maps_t = nc.dram_tensor("maps_pkb", [P, KTC], fdt, kind="ExternalInput")
    mask_t = nc.dram_tensor("sp_mask", [T, H, W], i32, kind="ExternalInput")
    wfc_t = nc.dram_tensor("W_fcT", [C, D_OUT], bf16, kind="ExternalInput")
    out_t = nc.dram_tensor("out", [T, N_SP, D_OUT], f32, kind="ExternalOutput")

    mask_r = mask_t.ap().rearrange("t h w -> t (h w)").rearrange(
        "t (k p) -> t k p", p=P
    )
    wfc_r = wfc_t.ap().rearrange("(cj c) d -> c cj d", c=P)

    with tile.TileContext(nc) as tc, ExitStack() as ctx:
        const_pool = ctx.enter_context(tc.tile_pool(name="const", bufs=1))
        feats_pool = ctx.enter_context(tc.tile_pool(name="feats", bufs=feats_bufs))
        mask_pool = ctx.enter_context(tc.tile_pool(name="mask", bufs=8))
        maskf_pool = ctx.enter_context(tc.tile_pool(name="maskf", bufs=8))
        oall_pool = ctx.enter_context(tc.tile_pool(name="oall", bufs=oall_bufs))
        segsb_pool = ctx.enter_context(tc.tile_pool(name="segsb", bufs=4))
        ctsb_pool = ctx.enter_context(tc.tile_pool(name="ctsb", bufs=4))
        segfs_pool = ctx.enter_context(tc.tile_pool(name="segfs", bufs=4))
        outsb_pool = ctx.enter_context(tc.tile_pool(name="outsb", bufs=2))
        small_pool = ctx.enter_context(tc.tile_pool(name="small", bufs=4))
        segc_pool = ctx.enter_context(
            tc.tile_pool(name="segc", bufs=1, space="PSUM")
        )
        seg_pool = ctx.enter_context(tc.tile_pool(name="seg", bufs=2, space="PSUM"))
        tp_pool = ctx.enter_context(tc.tile_pool(name="tp", bufs=2, space="PSUM"))

        # iota constant: column (k, s) holds value s (bf16: 0..99 exact)
        iota_tile = const_pool.tile([P, K * N_SP], bf16)
        nc.gpsimd.iota(
            iota_tile[:].rearrange("p (k s) -> p k s", s=N_SP),
            pattern=[[0, K], [1, N_SP]],
            base=0,
            channel_multiplier=0,
            allow_small_or_imprecise_dtypes=True,
        )

        identity = const_pool.tile([P, P], bf16)
        make_identity(nc, identity[:])

        eps_tile = const_pool.tile([P, 1], f32)
        nc.vector.memset(eps_tile[:], 1e-30)

        wt_tile = const_pool.tile([P, CB * D_OUT], bf16)
        nc.sync.dma_start(
            out=wt_tile[:].rearrange("c (cj d) -> c cj d", d=D_OUT),
            in_=wfc_r,
        )

        copy = nc.vector.tensor_copy if copy_eng == "vector" else nc.scalar.copy

        static_chunks = None
        if compute_only:
            static_chunks = []
            for cj in range(nchunk):
                sc = const_pool.tile([P, KPC * T * C], fdt, name=f"static{cj}")
                nc.sync.dma_start(
                    out=sc[:],
                    in_=maps_t.ap()[:, cj * KPC * T * C : (cj + 1) * KPC * T * C],
                )
                static_chunks.append(sc)

        def emit_prologue():
            """mask -> transpose -> one-hot for one rep; returns oall tiles."""
            oalls = []
            for t in range(T):
                mask_raw = mask_pool.tile([K, P], bf16)
                nc.gpsimd.dma_start(out=mask_raw[:], in_=mask_r[t])
                mask_ps = tp_pool.tile([P, K], bf16, name="tp")
                nc.tensor.transpose(
                    out=mask_ps[:], in_=mask_raw[:], identity=identity[:K, :K]
                )
                mask_f = maskf_pool.tile([P, K], bf16)
                nc.vector.tensor_copy(out=mask_f[:], in_=mask_ps[:])
                oall = oall_pool.tile([P, K * N_SP], fdt)
                nc.vector.tensor_tensor(
                    out=oall[:].rearrange("p (k s) -> p k s", s=N_SP),
                    in0=mask_f[:].to_broadcast([P, K, N_SP]),
                    in1=iota_tile[:].rearrange("p (k s) -> p k s", s=N_SP),
                    op=mybir.AluOpType.is_equal,
                )
                oalls.append(oall)
            return oalls

        oalls = None if dma_only else emit_prologue()

        for r in range(reps):
            if compute_only:
                chunks = static_chunks
            else:
                chunks = []
                for cj in range(nchunk):
                    ct = feats_pool.tile([P, KPC * T * C], fdt)
                    eng = nc.sync if cj % 2 == 0 else nc.scalar
                    eng.dma_start(
                        out=ct[:],
                        in_=maps_t.ap()[:, cj * KPC * T * C : (cj + 1) * KPC * T * C],
                    )
                    chunks.append(ct)

            if dma_only:
                probe = small_pool.tile([P, 1], f32)
                nc.vector.reduce_sum(
                    out=probe[:], in_=chunks[-1][:, :4], axis=mybir.AxisListType.X
                )
                continue

            cur_oalls = oalls
            if r + 1 < reps:
                oalls = emit_prologue()  # next rep's, built during this rep

            # stage A: fat matmuls, one-hot stationary, feats streamed N=512
            segcs = [
                segc_pool.tile([N_SP, C], f32, name=f"segc{t}")
                for t in range(T)
            ]
            for cj in range(nchunk):
                ct = chunks[cj]
                for t in range(T):
                    for kk in range(KPC):
                        k = cj * KPC + kk
                        nc.tensor.matmul(
                            out=segcs[t][:],
                            lhsT=cur_oalls[t][:, k * N_SP : (k + 1) * N_SP],
                            rhs=ct[:, kk * T * C + t * C : kk * T * C + (t + 1) * C],
                            start=(k == 0),
                            stop=(k == K - 1),
                        )

            # stage B (phase-major): PSUM->SBUF, 4 PE transposes per frame,
            # then project through W^T
            segsbs, ctsbs, segs, segfs_l = [], [], [], []
            for t in range(T):
                segc_sb = segsb_pool.tile([N_SP, C], bf16)
                copy(out=segc_sb[:], in_=segcs[t][:])
                segsbs.append(segc_sb)
            for t in range(T):
                ct_sb = ctsb_pool.tile([P, CB * N_SP], bf16)
                for cb in range(CB):
                    ctp = tp_pool.tile([P, N_SP], bf16, name="tp")
                    nc.tensor.transpose(
                        out=ctp[:],
                        in_=segsbs[t][:, cb * P : (cb + 1) * P],
                        identity=identity[:N_SP, :N_SP],
                    )
                    copy(out=ct_sb[:, cb * N_SP : (cb + 1) * N_SP], in_=ctp[:])
                ctsbs.append(ct_sb)
            for t in range(T):
                seg = seg_pool.tile([N_SP, D_OUT], f32)
                for cb in range(CB):
                    nc.tensor.matmul(
                        out=seg[:],
                        lhsT=ctsbs[t][:, cb * N_SP : (cb + 1) * N_SP],
                        rhs=wt_tile[:, cb * D_OUT : (cb + 1) * D_OUT],
                        start=(cb == 0),
                        stop=(cb == CB - 1),
                    )
                segs.append(seg)
            outsb = outsb_pool.tile([N_SP, T * D_OUT], f32)
            for t in range(T):
                seg_fs = segfs_pool.tile([N_SP, D_OUT], f32)
                nc.vector.tensor_copy(out=seg_fs[:], in_=segs[t][:])
                segfs_l.append(seg_fs)
                sq = small_pool.tile([N_SP, D_OUT], f32)
                ss = small_pool.tile([N_SP, 1], f32)
                nc.vector.tensor_mul(out=sq[:], in0=seg_fs[:], in1=seg_fs[:])
                nc.vector.reduce_sum(
                    out=ss[:], in_=sq[:], axis=mybir.AxisListType.X
                )
                nrm = small_pool.tile([N_SP, 1], f32)
                nc.scalar.activation(
                    out=nrm[:],
                    in_=ss[:],
                    func=mybir.ActivationFunctionType.Sqrt,
                    bias=eps_tile[:N_SP],
                )
                inv = small_pool.tile([N_SP, 1], f32)
                nc.vector.reciprocal(out=inv[:], in_=nrm[:])
                nc.vector.tensor_scalar_mul(
                    out=outsb[:, t * D_OUT : (t + 1) * D_OUT],
                    in0=segfs_l[t][:],
                    scalar1=inv[:],
                )
            nc.gpsimd.dma_start(
                out=out_t.ap().rearrange("t s d -> s t d"),
                in_=outsb[:].rearrange("s (t d) -> s t d", d=D_OUT),
            )

    nc.compile()
    return nc


VARIANT = "pkbf"


def _get_program():
    if "nc" not in _CACHE:
        if VARIANT == "pm":
            _CACHE["nc"] = _build_program_pm()
        elif VARIANT == "pkb":
            _CACHE["nc"] = _build_program_pkb()
        elif VARIANT == "pkb8":
            _CACHE["nc"] = _build_program_pkb(fp8=True)
        elif VARIANT == "pkbf":
            _CACHE["nc"] = _build_program_pkbf()
        else:
            _CACHE["nc"] = _build_program()
    return _CACHE["nc"]


def _to_pkb(maps_q):
    """(B,C,T,H,W) -> (B, P, K*T*C) with column layout (k, t, c)."""
    return np.ascontiguousarray(
        maps_q.reshape(B, C, T, HW // P, P)
        .transpose(0, 4, 3, 2, 1)
        .reshape(B, P, (HW // P) * T * C)
    )


def _prep_in_maps(maps, sp_mask, W_fc):
    """Host-side input prep (dtype cast + layout) for the current VARIANT."""
    import ml_dtypes

    bf16 = ml_dtypes.bfloat16
    maps = np.asarray(maps, dtype=np.float32)
    sp_mask = np.asarray(sp_mask, dtype=np.int32)
    W_fc = np.asarray(W_fc, dtype=np.float32)
    assert maps.shape == (B, C, T, H, W)
    wt = np.ascontiguousarray(W_fc.T).astype(bf16)  # (C, D_OUT)
    if VARIANT == "pm":
        maps_pm = np.ascontiguousarray(
            maps.astype(bf16).transpose(0, 2, 3, 4, 1).reshape(B, T, HW, C)
        )
        return [
            {"maps_pm": maps_pm[b], "sp_mask": sp_mask[b], "W_fcT": wt}
            for b in range(B)
        ]
    if VARIANT in ("pkb", "pkb8", "pkbf"):
        fdt = bf16 if VARIANT == "pkb" else ml_dtypes.float8_e3m4
        maps_pkb = _to_pkb(maps.astype(fdt))
        return [
            {"maps_pkb": maps_pkb[b], "sp_mask": sp_mask[b], "W_fcT": wt}
            for b in range(B)
        ]
    return [
        {"maps_bf16": maps[b].astype(bf16), "sp_mask": sp_mask[b], "W_fcT": wt}
        for b in range(B)
    ]


def kernel(maps, sp_mask, W_fc, max_sp_num):
    from concourse.bass_utils import run_bass_kernel_spmd

    assert int(max_sp_num) == N_SP
    in_maps = _prep_in_maps(maps, sp_mask, W_fc)
    nc = _get_program()
    res = run_bass_kernel_spmd(nc, in_maps, core_ids=list(range(N_CORES)))
    # per-core out is (T, N_SP, D_OUT); full output is (B, D_OUT, T, N_SP)
    out = np.stack([res.results[b]["out"] for b in range(B)], axis=0)
    return np.ascontiguousarray(out.transpose(0, 3, 1, 2)).astype(np.float32)

